# revision 35
# baseline (speedup 1.0000x reference)
"""Single-head causal attention (B=4, T=4096, C=1024, H=64) on 8 TRN2 cores.

Sharding: 2 cores per batch element, query rows split between the pair for
causal balance. Key-block EMISSION order is a per-fold permutation chosen so
that the q-block of attention slot s sits at emission position 2s on BOTH
folds; the q projection then piggybacks on the k/v projection of that block
(same x tile, no separate x_q DMA). Slot s attends emission blocks 0..2s+1:
blocks below the band are fully causal-valid by construction, emission block
2s is the true diagonal (static triangle mask, additive pre-exp), and block
2s+1 is a filler that is fully valid or fully dead per fold - zeroed for free
via a data-driven per-partition bias on the exp activation (exp(x/8 - 100)).

K and V projections are packed into one matmul ([Wv | Wk] stationary, 128
wide): v^T lands on PSUM partitions 0-63, k^T on 64-127, one bias-add writes
both into a combined vkT tile. Scores run as row-tiled pairs (PE rows 0-63 /
64-127 concurrently). The weights@V matmul uses fp8e4 DoubleRow (two key
chunks contracted per pass, K=256 virtual). A ones-column on v gives the
softmax denominator; normalization and the final transpose happen on host.
"""

import numpy as np
import ml_dtypes

import concourse.bacc as bacc
import concourse.mybir as mybir
from concourse.tile import TileContext
from concourse.masks import make_identity
from concourse.bass_utils import run_bass_kernel_spmd

B, T, C, H = 4, 4096, 1024, 64
P = 128                     # SBUF partitions
NB = T // P                 # 32 key chunks of 128
CB = C // P                 # 8 contraction chunks of 128
QS = 512                    # query/projection block width
TB = T // QS                # 8 key-side projection blocks
NSLOT = 4                   # query slots per core (2048 queries)
HE = H + 1                  # v extended with a ones column (softmax denom)
VP = 80                     # vext pair-stride padding (fp8 step % 16 == 0)

USE_DR = True               # fp8 DoubleRow for the weights@V matmul

# emission position -> key block; q-block of slot s is at position 2s on
# both folds, and position 2s+1 is the other fold's q-block (the filler).
EM_PERM = [
    [1, 0, 2, 3, 4, 5, 7, 6],   # fold 0
    [0, 1, 3, 2, 5, 4, 6, 7],   # fold 1
]
FOLD_SLOT_QSTART = [[p[2 * s] * QS for s in range(NSLOT)] for p in EM_PERM]

F32 = mybir.dt.float32
BF16 = mybir.dt.bfloat16
FP8 = mybir.dt.float8e4
EDT = FP8 if USE_DR else BF16
BF16NP = ml_dtypes.bfloat16


def build_bass():
    nc = bacc.Bacc("TRN2", target_bir_lowering=False, debug=False)

    x_d = nc.declare_dram_parameter("x_all", [TB, P, CB, QS], BF16, isOutput=False)
    wvk_d = nc.declare_dram_parameter("w_vk", [P, CB, P], BF16, isOutput=False)
    wq_d = nc.declare_dram_parameter("w_q", [P, CB, P], BF16, isOutput=False)
    # col 0: [bv; bk], col 1: [bq; 0], cols 2-5: per-slot filler exp-bias
    prm_d = nc.declare_dram_parameter("prm", [P, 8], F32, isOutput=False)
    out_d = nc.declare_dram_parameter("out", [NSLOT, HE, QS], F32, isOutput=True)

    with TileContext(nc) as tc:
        with (
            tc.tile_pool(name="const", bufs=1) as const,
            tc.tile_pool(name="xio", bufs=3) as xio,
            tc.tile_pool(name="work", bufs=3) as work,
            tc.tile_pool(name="wout", bufs=2) as wout,
            tc.tile_pool(name="ps_s", bufs=2, space="PSUM") as ps_s,
            tc.tile_pool(name="ps_o", bufs=2, space="PSUM") as ps_o,
            tc.tile_pool(name="ps_p", bufs=1, space="PSUM") as ps_p,
            tc.tile_pool(name="ps_x", bufs=1, space="PSUM") as ps_x,
        ):
            # ---- persistent SBUF state; DMA order = head critical path ----
            wvk_sb = const.tile([P, CB, P], BF16, tag="wvk")
            nc.sync.dma_start(wvk_sb[:], wvk_d[:])
            wq_sb = const.tile([P, CB, P], BF16, tag="wq")
            prm_sb = const.tile([P, 8], F32, tag="prm")

            # force the exp table-set load (~2.7us) off the critical path
            warm = const.tile([P, 1], F32, tag="warm")
            nc.vector.memset(warm[:], 0.0)
            nc.scalar.activation(warm[:], warm[:],
                                 mybir.ActivationFunctionType.Exp)

            vkT = const.tile([P, T], BF16, tag="vkT")      # v^T low / k^T high
            kTlow = const.tile([H, T], BF16, tag="kTl")    # k^T copy, parts 0-63
            qTd = const.tile([P, NSLOT * QS], BF16, tag="qTd")  # q^T dup halves
            vext = const.tile([P, NB // 2, 2, VP], EDT, tag="vext")
            nc.vector.memset(vext[:, :, :, H:HE], 1.0)
            # slot 0 queries attend few keys: fp8 v-quantization doesn't
            # average out there, so slot 0 runs bf16 wv on chunks 0-7
            if USE_DR:
                vext_bf = const.tile([P, 4, 2, VP], BF16, tag="vextbf")
                nc.vector.memset(vext_bf[:, :, :, H:HE], 1.0)
            else:
                vext_bf = vext

            id64 = const.tile([H, H], BF16, tag="id64")
            make_identity(nc, id64[:])

            # static causal triangle for the diagonal 512-block (additive,
            # pre-exp): tri[p, c, qi] = 0 if qi >= c*128 + p else -1e4
            tri = const.tile([P, 4, QS], F32, tag="tri")
            nc.gpsimd.memset(tri[:], 0.0)
            for c in range(4):
                nc.gpsimd.affine_select(
                    out=tri[:, c, :], in_=tri[:, c, :],
                    compare_op=mybir.AluOpType.is_ge,
                    fill=-1e4, base=-c * P,
                    pattern=[[1, QS]], channel_multiplier=-1,
                )

            tri01 = const.tile([P, 4, QS], BF16, tag="tri01")
            nc.gpsimd.memset(tri01[:], 1.0)
            for c in range(4):
                nc.gpsimd.affine_select(
                    out=tri01[:, c, :], in_=tri01[:, c, :],
                    compare_op=mybir.AluOpType.is_ge,
                    fill=0.0, base=-c * P,
                    pattern=[[1, QS]], channel_multiplier=-1,
                )

            # ---- emission thunks (PE is in-order; emission order is the
            # static schedule). Each key block: 2 half DMAs, 8 packed [Wv|Wk]
            # matmuls, one bias-add writing v^T+k^T, k^T low copy, optional q
            # projection (emission positions 0,2,4,6), 4 v transposes. ----
            def kv_thunks(em):
                st = {}
                cols = slice(em * QS, (em + 1) * QS)

                def mk_load(hf):
                    def f():
                        if hf == 0:
                            st["xt"] = xio.tile([P, CB, QS], BF16, tag="xt", name="xt")
                        nc.sync.dma_start(
                            st["xt"][:, 2 * hf : 2 * hf + 2, :],
                            x_d[em, :, 2 * hf : 2 * hf + 2, :],
                        )
                    return f

                def mk_mm(c):
                    def f():
                        if c == 0:
                            st["vk"] = ps_p.tile([P, QS], F32, tag="pp", name="vk")
                        nc.tensor.matmul(
                            st["vk"][:], wvk_sb[:, c, :], st["xt"][:, c, :],
                            start=(c == 0), stop=(c == CB - 1),
                        )
                    return f

                def bias():
                    nc.vector.tensor_scalar_add(
                        vkT[:, cols], st["vk"][:], prm_sb[:, 0:1]
                    )

                def ktcopy():
                    # em0/em1 ride the idle scalar ring (their biases finish
                    # before any exp queues behind them); later blocks use
                    # gpsimd software-DGE, off the x rings and the ACT queue
                    if em < 2:
                        nc.scalar.dma_start(kTlow[:, cols], vkT[H:, cols])
                    else:
                        nc.gpsimd.dma_start(kTlow[:, cols], vkT[H:, cols])

                def mk_qmm(c):
                    def f():
                        if c == 0:
                            st["q"] = ps_x.tile([P, QS], F32, tag="px", name="q")
                        nc.tensor.matmul(
                            st["q"][:], wq_sb[:, c, :], st["xt"][:, c, :],
                            start=(c == 0), stop=(c == CB - 1),
                        )
                    return f

                qcols = slice((em // 2) * QS, (em // 2 + 1) * QS)

                def qbias():
                    nc.vector.tensor_scalar_add(
                        qTd[:, qcols], st["q"][:], prm_sb[:, 1:2]
                    )

                def mk_vtr(s):
                    def f():
                        tk = 4 * em + s
                        vtp = ps_x.tile([P, H], BF16, tag="px", name="vtp")
                        nc.tensor.transpose(
                            vtp[:], vkT[:H, tk * P : (tk + 1) * P], id64[:]
                        )
                        nc.vector.tensor_copy(
                            vext[:, tk // 2, tk % 2, :H], vtp[:]
                        )
                        if USE_DR and em < 2:
                            nc.vector.tensor_copy(
                                vext_bf[:, tk // 2, tk % 2, :H], vtp[:]
                            )
                    return f

                th = [mk_load(h) for h in range(4)]
                proj = [mk_mm(c) for c in range(CB)] + [bias, ktcopy]
                if em % 2 == 0:
                    qproj = [mk_qmm(c) for c in range(CB)] + [qbias]
                    # fill-B blocks (em = 2s+2): their q gates the next
                    # slot's first scores - project q before k/v there
                    th += qproj + proj if em >= 2 else proj + qproj
                else:
                    th += proj
                th += [mk_vtr(s) for s in range(4)]
                return th

            # preamble: emission block 0 computes; x1 loads are hoisted
            # so the DMA rings stay saturated ahead of their consumers
            th_em = [kv_thunks(em) for em in range(TB)]
            for t in th_em[0][:4]:
                t()
            nc.sync.dma_start(wq_sb[:], wq_d[:])
            nc.sync.dma_start(prm_sb[:], prm_d[:])
            # dummy matmuls on garbage data warm the PE HAM clock gate
            # (4/8 -> 8/8) while the first x quarters are still in flight
            wps = ps_s.tile([P, 2, QS], F32, tag="sT", name="wps")
            for _ in range(10):
                nc.tensor.matmul(
                    wps[:, 0, :], qTd[:H, :P], qTd[:H, :QS],
                    start=True, stop=True,
                )
            for t in th_em[1][:4]:
                t()
            for t in th_em[0][4:]:
                t()

            def parts(em):
                th = th_em[em]
                if em == 0:
                    return th[0:4], th[4:14], th[14:23], th[23:27]
                if em % 2 == 0:
                    return th[0:4], th[13:23], th[4:13], th[23:27]
                return th[0:4], th[4:14], [], th[14:18]

            PR = [parts(em) for em in range(TB)]  # L, K, Q, V per block

            # cross-slot pair stream: slot s+1's pairs start as soon as its
            # q projection (block 2s+2) is done; projection blobs are
            # injected at their exact dependency points instead of per-slot
            # fill windows, so the exp stream never waits a slot boundary
            def TH(*ems_parts):
                return [("th", [t for em, pi in ems_parts
                                for t in PR[em][pi]])]

            SCHED = (
                [("pair", 0, 0), ("pair", 0, 1)]
                + TH((1, 1), (1, 3))                      # K1 V1
                + [("pair", 0, 2), ("pair", 0, 3)]
                + TH((2, 0), (2, 2))                      # L2 Q2
                + [("pair", 1, 2), ("pair", 1, 3), ("pair", 1, 0),
                   ("pair", 1, 1)]
                + TH((2, 1), (2, 3), (3, 0))              # K2 V2 L3
                + [("pair", 1, 4), ("pair", 1, 5)]
                + TH((3, 1), (3, 3))                      # K3 V3
                + [("pair", 1, 6), ("pair", 1, 7)]
                + TH((4, 0), (4, 2))                      # L4 Q4
                + [("pair", 2, 2), ("pair", 2, 3), ("pair", 2, 4),
                   ("pair", 2, 5), ("pair", 2, 0), ("pair", 2, 1)]
                + TH((4, 1), (4, 3), (5, 0))              # K4 V4 L5
                + [("pair", 2, 6), ("pair", 2, 7), ("pair", 2, 8),
                   ("pair", 2, 9)]
                + TH((5, 1), (5, 3))                      # K5 V5
                + [("pair", 2, 10), ("pair", 2, 11)]
                + TH((6, 0), (6, 2))                      # L6 Q6
                + [("pair", 3, 2), ("pair", 3, 3), ("pair", 3, 4),
                   ("pair", 3, 5), ("pair", 3, 6), ("pair", 3, 7),
                   ("pair", 3, 0), ("pair", 3, 1)]
                + TH((6, 1), (6, 3), (7, 0))              # K6 V6 L7
                + [("pair", 3, 8), ("pair", 3, 9), ("pair", 3, 10),
                   ("pair", 3, 11)]
                + TH((7, 1), (7, 3))                      # K7 V7
                + [("pair", 3, 12), ("pair", 3, 13), ("pair", 3, 14),
                   ("pair", 3, 15)]
            )

            oaccs = {}
            emitted = [0] * NSLOT
            pipe = []        # exps awaiting their wv matmul (lag 1)
            pending = None   # (oacc, slot) whose copy+store is deferred

            def flush_out(p):
                oacc, sl = p
                oT = wout.tile([HE, QS], F32, tag="oT", name="oT")
                nc.vector.tensor_copy(oT[:], oacc[:])
                nc.sync.dma_start(out_d[sl], oT[:])

            def emit_wv(expT, slot, tkp, first, last):
                if USE_DR and slot > 0:
                    nc.tensor.matmul(
                        oaccs[slot][:], vext[:, tkp, :, :HE], expT[:],
                        start=first, stop=last,
                        perf_mode=mybir.MatmulPerfMode.DoubleRow,
                    )
                else:
                    for h in range(2):
                        nc.tensor.matmul(
                            oaccs[slot][:], vext_bf[:, tkp, h, :HE],
                            expT[:, h, :],
                            start=(first and h == 0),
                            stop=(last and h == 1),
                        )

            def pop_wv():
                nonlocal pending
                e = pipe.pop(0)
                emit_wv(*e)
                if e[4]:  # slot's last wv: its output copy can be queued
                    if pending is not None:
                        flush_out(pending)
                    pending = (oaccs[e[1]], e[1])

            for ent in SCHED:
                if ent[0] == "th":
                    for t in ent[1]:
                        t()
                    continue
                _, slot, tkp = ent
                if slot not in oaccs:
                    oaccs[slot] = ps_o.tile([HE, QS], F32, tag="oacc",
                                            name="oacc")
                npairs = 4 * (slot + 1)
                qcols = slice(slot * QS, (slot + 1) * QS)
                use_dr = USE_DR and slot > 0
                sps = ps_s.tile([P, 2, QS], F32, tag="sT")
                for h in range(2):
                    tk = 2 * tkp + h
                    if h == 0:
                        nc.tensor.matmul(
                            sps[:, 0, :], kTlow[:, tk * P : (tk + 1) * P],
                            qTd[:H, qcols], start=True, stop=True,
                        )
                    else:
                        nc.tensor.matmul(
                            sps[:, 1, :], vkT[H:, tk * P : (tk + 1) * P],
                            qTd[H:, qcols], start=True, stop=True,
                        )
                j = tkp - 4 * slot
                if j in (0, 1) and slot > 0:
                    # diagonal triangle, additive pre-exp (overlaps exps)
                    nc.vector.tensor_tensor(
                        sps[:], sps[:], tri[:, 2 * j : 2 * j + 2, :],
                        mybir.AluOpType.add,
                    )
                expT = work.tile([P, 2, QS], EDT if use_dr else BF16,
                                 tag="expT")
                # filler block (j in 2,3): dead folds zeroed via exp bias
                fb = prm_sb[:, 2 + slot : 3 + slot] if j in (2, 3) else 0.0
                nc.scalar.activation(
                    expT[:], sps[:], mybir.ActivationFunctionType.Exp,
                    scale=float(H) ** -0.5, bias=fb,
                )
                if j in (0, 1) and slot == 0:
                    # slot 0 masks after exp: the first exps of the kernel
                    # sit on the serial head path, the DVE add would delay
                    nc.vector.tensor_tensor(
                        expT[:], expT[:], tri01[:, 2 * j : 2 * j + 2, :],
                        mybir.AluOpType.mult,
                    )
                pipe.append((expT, slot, tkp, emitted[slot] == 0,
                             emitted[slot] == npairs - 1))
                emitted[slot] += 1
                if len(pipe) > 1:
                    pop_wv()
            while pipe:
                pop_wv()
            flush_out(pending)

    nc.compile()
    return nc


_NC_CACHE = None


def _get_nc():
    global _NC_CACHE
    if _NC_CACHE is None:
        _NC_CACHE = build_bass()
    return _NC_CACHE


def _core_inputs(x, Wq, bq, Wk, bk, Wv, bv, b, fold):
    xT = np.asarray(x[b], dtype=np.float32).T       # [C, T] view
    perm = EM_PERM[fold]
    xa = np.empty((TB, P, CB, QS), dtype=BF16NP)
    for em, blk in enumerate(perm):
        t = xT[:, blk * QS : (blk + 1) * QS].reshape(CB, P, QS)
        xa[em] = t.transpose(1, 0, 2)

    wv = np.asarray(Wv, np.float32).reshape(CB, P, H)
    wk = np.asarray(Wk, np.float32).reshape(CB, P, H)
    w_vk = np.concatenate([wv, wk], axis=2).transpose(1, 0, 2)  # [P, CB, 128]
    wq1 = np.asarray(Wq, np.float32).reshape(CB, P, H)
    w_q = np.concatenate([wq1, wq1], axis=2).transpose(1, 0, 2)

    prm = np.zeros((P, 8), dtype=np.float32)
    prm[:H, 0] = np.asarray(bv, np.float32)
    prm[H:, 0] = np.asarray(bk, np.float32)
    prm[:H, 1] = np.asarray(bq, np.float32)
    prm[H:, 1] = np.asarray(bq, np.float32)
    for s in range(NSLOT):
        if perm[2 * s + 1] > perm[2 * s]:   # filler block is fully dead
            prm[:, 2 + s] = -100.0
    return {
        "x_all": np.ascontiguousarray(xa),
        "w_vk": np.ascontiguousarray(w_vk.astype(BF16NP)),
        "w_q": np.ascontiguousarray(w_q.astype(BF16NP)),
        "prm": prm,
    }


def _unshard(results):
    out = np.empty((B, T, H), dtype=np.float32)
    for core in range(8):
        b, fold = core // 2, core % 2
        o = results[core]["out"]            # [NSLOT, 65, 512]
        for s, q0 in enumerate(FOLD_SLOT_QSTART[fold]):
            out[b, q0 : q0 + QS, :] = (o[s, :H, :] / o[s, H : H + 1, :]).T
    return out


def kernel(x, Wq, bq, Wk, bk, Wv, bv):
    x = np.asarray(x, dtype=np.float32)
    nc = _get_nc()
    core_ids = list(range(8))
    in_maps = [
        _core_inputs(x, Wq, bq, Wk, bk, Wv, bv, core // 2, core % 2)
        for core in core_ids
    ]
    res = run_bass_kernel_spmd(nc, in_maps, core_ids)
    return _unshard(res.results)


# revision 36
# speedup vs baseline: 1.0222x; 1.0222x over previous
"""Single-head causal attention (B=4, T=4096, C=1024, H=64) on 8 TRN2 cores.

Sharding: 2 cores per batch element, query rows split between the pair for
causal balance. Key-block EMISSION order is a per-fold permutation chosen so
that the q-block of attention slot s sits at emission position 2s on BOTH
folds; the q projection then piggybacks on the k/v projection of that block
(same x tile, no separate x_q DMA). Slot s attends emission blocks 0..2s+1:
blocks below the band are fully causal-valid by construction, emission block
2s is the true diagonal (static triangle mask, additive pre-exp), and block
2s+1 is a filler that is fully valid or fully dead per fold - zeroed for free
via a data-driven per-partition bias on the exp activation (exp(x/8 - 100)).

K and V projections are packed into one matmul ([Wv | Wk] stationary, 128
wide): v^T lands on PSUM partitions 0-63, k^T on 64-127, one bias-add writes
both into a combined vkT tile. Scores run as row-tiled pairs (PE rows 0-63 /
64-127 concurrently). The weights@V matmul uses fp8e4 DoubleRow (two key
chunks contracted per pass, K=256 virtual). A ones-column on v gives the
softmax denominator; normalization and the final transpose happen on host.
"""

import numpy as np
import ml_dtypes

import concourse.bacc as bacc
import concourse.mybir as mybir
from concourse.tile import TileContext
from concourse.masks import make_identity
from concourse.bass_utils import run_bass_kernel_spmd

B, T, C, H = 4, 4096, 1024, 64
P = 128                     # SBUF partitions
NB = T // P                 # 32 key chunks of 128
CB = C // P                 # 8 contraction chunks of 128
QS = 512                    # query/projection block width
TB = T // QS                # 8 key-side projection blocks
NSLOT = 4                   # query slots per core (2048 queries)
HE = H + 1                  # v extended with a ones column (softmax denom)
VP = 80                     # vext pair-stride padding (fp8 step % 16 == 0)

USE_DR = True               # fp8 DoubleRow for the weights@V matmul

# emission position -> key block; q-block of slot s is at position 2s on
# both folds, and position 2s+1 is the other fold's q-block (the filler).
EM_PERM = [
    [1, 0, 2, 3, 4, 5, 7, 6],   # fold 0
    [0, 1, 3, 2, 5, 4, 6, 7],   # fold 1
]
FOLD_SLOT_QSTART = [[p[2 * s] * QS for s in range(NSLOT)] for p in EM_PERM]

F32 = mybir.dt.float32
BF16 = mybir.dt.bfloat16
FP8 = mybir.dt.float8e4
EDT = FP8 if USE_DR else BF16
BF16NP = ml_dtypes.bfloat16


def build_bass():
    nc = bacc.Bacc("TRN2", target_bir_lowering=False, debug=False)

    x_d = nc.declare_dram_parameter("x_all", [TB, P, CB, QS], BF16, isOutput=False)
    wvk_d = nc.declare_dram_parameter("w_vk", [P, CB, P], BF16, isOutput=False)
    wq_d = nc.declare_dram_parameter("w_q", [P, CB, P], BF16, isOutput=False)
    # col 0: [bv; bk], col 1: [bq; 0], cols 2-5: per-slot filler exp-bias
    prm_d = nc.declare_dram_parameter("prm", [P, 8], F32, isOutput=False)
    out_d = nc.declare_dram_parameter("out", [NSLOT, HE, QS], F32, isOutput=True)

    with TileContext(nc) as tc:
        with (
            tc.tile_pool(name="const", bufs=1) as const,
            tc.tile_pool(name="xio", bufs=3) as xio,
            tc.tile_pool(name="work", bufs=3) as work,
            tc.tile_pool(name="wout", bufs=2) as wout,
            tc.tile_pool(name="ps_s", bufs=2, space="PSUM") as ps_s,
            tc.tile_pool(name="ps_o", bufs=2, space="PSUM") as ps_o,
            tc.tile_pool(name="ps_p", bufs=1, space="PSUM") as ps_p,
            tc.tile_pool(name="ps_x", bufs=1, space="PSUM") as ps_x,
        ):
            # ---- persistent SBUF state; DMA order = head critical path ----
            wvk_sb = const.tile([P, CB, P], BF16, tag="wvk")
            nc.sync.dma_start(wvk_sb[:], wvk_d[:])
            wq_sb = const.tile([P, CB, P], BF16, tag="wq")
            prm_sb = const.tile([P, 8], F32, tag="prm")

            # force the exp table-set load (~2.7us) off the critical path
            warm = const.tile([P, 1], F32, tag="warm")
            nc.vector.memset(warm[:], 0.0)
            nc.scalar.activation(warm[:], warm[:],
                                 mybir.ActivationFunctionType.Exp)

            vkT = const.tile([P, T], BF16, tag="vkT")      # v^T low / k^T high
            kTlow = const.tile([H, T], BF16, tag="kTl")    # k^T copy, parts 0-63
            qTd = const.tile([P, NSLOT * QS], BF16, tag="qTd")  # q^T dup halves
            vext = const.tile([P, NB // 2, 2, VP], EDT, tag="vext")
            nc.vector.memset(vext[:, :, :, H:HE], 1.0)
            # slot 0 queries attend few keys: fp8 v-quantization doesn't
            # average out there, so slot 0 runs bf16 wv on chunks 0-7
            if USE_DR:
                vext_bf = const.tile([P, 4, 2, VP], BF16, tag="vextbf")
                nc.vector.memset(vext_bf[:, :, :, H:HE], 1.0)
            else:
                vext_bf = vext

            id64 = const.tile([H, H], BF16, tag="id64")
            make_identity(nc, id64[:])

            # static causal triangle for the diagonal 512-block (additive,
            # pre-exp): tri[p, c, qi] = 0 if qi >= c*128 + p else -1e4
            tri = const.tile([P, 4, QS], F32, tag="tri")
            nc.gpsimd.memset(tri[:], 0.0)
            for c in range(4):
                nc.gpsimd.affine_select(
                    out=tri[:, c, :], in_=tri[:, c, :],
                    compare_op=mybir.AluOpType.is_ge,
                    fill=-1e4, base=-c * P,
                    pattern=[[1, QS]], channel_multiplier=-1,
                )

            tri01 = const.tile([P, 4, QS], BF16, tag="tri01")
            nc.gpsimd.memset(tri01[:], 1.0)
            for c in range(4):
                nc.gpsimd.affine_select(
                    out=tri01[:, c, :], in_=tri01[:, c, :],
                    compare_op=mybir.AluOpType.is_ge,
                    fill=0.0, base=-c * P,
                    pattern=[[1, QS]], channel_multiplier=-1,
                )

            # ---- emission thunks (PE is in-order; emission order is the
            # static schedule). Each key block: 2 half DMAs, 8 packed [Wv|Wk]
            # matmuls, one bias-add writing v^T+k^T, k^T low copy, optional q
            # projection (emission positions 0,2,4,6), 4 v transposes. ----
            def kv_thunks(em):
                st = {}
                cols = slice(em * QS, (em + 1) * QS)

                def mk_load(hf):
                    def f():
                        if hf == 0:
                            st["xt"] = xio.tile([P, CB, QS], BF16, tag="xt", name="xt")
                        nc.sync.dma_start(
                            st["xt"][:, 2 * hf : 2 * hf + 2, :],
                            x_d[em, :, 2 * hf : 2 * hf + 2, :],
                        )
                    return f

                def mk_mm(c):
                    def f():
                        if c == 0:
                            st["vk"] = ps_p.tile([P, QS], F32, tag="pp", name="vk")
                        nc.tensor.matmul(
                            st["vk"][:], wvk_sb[:, c, :], st["xt"][:, c, :],
                            start=(c == 0), stop=(c == CB - 1),
                        )
                    return f

                def bias():
                    nc.vector.tensor_scalar_add(
                        vkT[:, cols], st["vk"][:], prm_sb[:, 0:1]
                    )

                def ktcopy():
                    # head blocks ride the sync ring right behind the x they
                    # depend on; later ones use the gpsimd software-DGE to
                    # stay off the x rings and off the strict-FIFO ACT queue
                    if em < 2:
                        nc.sync.dma_start(kTlow[:, cols], vkT[H:, cols])
                    else:
                        nc.gpsimd.dma_start(kTlow[:, cols], vkT[H:, cols])

                def mk_qmm(c):
                    def f():
                        if c == 0:
                            st["q"] = ps_x.tile([P, QS], F32, tag="px", name="q")
                        nc.tensor.matmul(
                            st["q"][:], wq_sb[:, c, :], st["xt"][:, c, :],
                            start=(c == 0), stop=(c == CB - 1),
                        )
                    return f

                qcols = slice((em // 2) * QS, (em // 2 + 1) * QS)

                def qbias():
                    nc.vector.tensor_scalar_add(
                        qTd[:, qcols], st["q"][:], prm_sb[:, 1:2]
                    )

                def mk_vtr(s):
                    def f():
                        tk = 4 * em + s
                        vtp = ps_x.tile([P, H], BF16, tag="px", name="vtp")
                        nc.tensor.transpose(
                            vtp[:], vkT[:H, tk * P : (tk + 1) * P], id64[:]
                        )
                        nc.vector.tensor_copy(
                            vext[:, tk // 2, tk % 2, :H], vtp[:]
                        )
                        if USE_DR and em < 2:
                            nc.vector.tensor_copy(
                                vext_bf[:, tk // 2, tk % 2, :H], vtp[:]
                            )
                    return f

                th = [mk_load(h) for h in range(4)]
                proj = [mk_mm(c) for c in range(CB)] + [bias, ktcopy]
                if em % 2 == 0:
                    qproj = [mk_qmm(c) for c in range(CB)] + [qbias]
                    # fill-B blocks (em = 2s+2): their q gates the next
                    # slot's first scores - project q before k/v there
                    th += qproj + proj if em >= 2 else proj + qproj
                else:
                    th += proj
                th += [mk_vtr(s) for s in range(4)]
                return th

            # preamble: emission block 0 computes; x1 loads are hoisted
            # so the DMA rings stay saturated ahead of their consumers
            th_em = [kv_thunks(em) for em in range(TB)]
            for t in th_em[0][:4]:
                t()
            nc.sync.dma_start(wq_sb[:], wq_d[:])
            nc.sync.dma_start(prm_sb[:], prm_d[:])
            # dummy matmuls on garbage data warm the PE HAM clock gate
            # (4/8 -> 8/8) while the first x quarters are still in flight
            wps = ps_s.tile([P, 2, QS], F32, tag="sT", name="wps")
            for _ in range(10):
                nc.tensor.matmul(
                    wps[:, 0, :], qTd[:H, :P], qTd[:H, :QS],
                    start=True, stop=True,
                )
            for t in th_em[1][:4]:
                t()
            for t in th_em[0][4:]:
                t()

            def parts(em):
                th = th_em[em]
                if em == 0:
                    return th[0:4], th[4:14], th[14:23], th[23:27]
                if em % 2 == 0:
                    return th[0:4], th[13:23], th[4:13], th[23:27]
                return th[0:4], th[4:14], [], th[14:18]

            PR = [parts(em) for em in range(TB)]  # L, K, Q, V per block

            # cross-slot pair stream: slot s+1's pairs start as soon as its
            # q projection (block 2s+2) is done; projection blobs are
            # injected at their exact dependency points instead of per-slot
            # fill windows, so the exp stream never waits a slot boundary
            def TH(*ems_parts):
                return [("th", [t for em, pi in ems_parts
                                for t in PR[em][pi]])]

            SCHED = (
                [("pair", 0, 0), ("pair", 0, 1)]
                + TH((1, 1), (1, 3))                      # K1 V1
                + [("pair", 0, 2), ("pair", 0, 3)]
                + TH((2, 0), (2, 2))                      # L2 Q2
                + [("pair", 1, 2), ("pair", 1, 3), ("pair", 1, 0),
                   ("pair", 1, 1)]
                + TH((2, 1), (2, 3), (3, 0))              # K2 V2 L3
                + [("pair", 1, 4), ("pair", 1, 5)]
                + TH((3, 1), (3, 3))                      # K3 V3
                + [("pair", 1, 6), ("pair", 1, 7)]
                + TH((4, 0), (4, 2))                      # L4 Q4
                + [("pair", 2, 2), ("pair", 2, 3), ("pair", 2, 4),
                   ("pair", 2, 5), ("pair", 2, 0), ("pair", 2, 1)]
                + TH((4, 1), (4, 3), (5, 0))              # K4 V4 L5
                + [("pair", 2, 6), ("pair", 2, 7), ("pair", 2, 8),
                   ("pair", 2, 9)]
                + TH((5, 1), (5, 3))                      # K5 V5
                + [("pair", 2, 10), ("pair", 2, 11)]
                + TH((6, 0), (6, 2))                      # L6 Q6
                + [("pair", 3, 2), ("pair", 3, 3), ("pair", 3, 4),
                   ("pair", 3, 5), ("pair", 3, 6), ("pair", 3, 7),
                   ("pair", 3, 0), ("pair", 3, 1)]
                + TH((6, 1), (6, 3), (7, 0))              # K6 V6 L7
                + [("pair", 3, 8), ("pair", 3, 9), ("pair", 3, 10),
                   ("pair", 3, 11)]
                + TH((7, 1), (7, 3))                      # K7 V7
                + [("pair", 3, 12), ("pair", 3, 13), ("pair", 3, 14),
                   ("pair", 3, 15)]
            )

            oaccs = {}
            emitted = [0] * NSLOT
            pipe = []        # exps awaiting their wv matmul (lag 1)
            pending = None   # (oacc, slot) whose copy+store is deferred

            def flush_out(p):
                oacc, sl = p
                oT = wout.tile([HE, QS], F32, tag="oT", name="oT")
                nc.vector.tensor_copy(oT[:], oacc[:])
                nc.sync.dma_start(out_d[sl], oT[:])

            def emit_wv(expT, slot, tkp, first, last):
                if USE_DR and slot > 0:
                    nc.tensor.matmul(
                        oaccs[slot][:], vext[:, tkp, :, :HE], expT[:],
                        start=first, stop=last,
                        perf_mode=mybir.MatmulPerfMode.DoubleRow,
                    )
                else:
                    for h in range(2):
                        nc.tensor.matmul(
                            oaccs[slot][:], vext_bf[:, tkp, h, :HE],
                            expT[:, h, :],
                            start=(first and h == 0),
                            stop=(last and h == 1),
                        )

            def pop_wv():
                nonlocal pending
                e = pipe.pop(0)
                emit_wv(*e)
                if e[4]:  # slot's last wv: its output copy can be queued
                    if pending is not None:
                        flush_out(pending)
                    pending = (oaccs[e[1]], e[1])

            for ent in SCHED:
                if ent[0] == "th":
                    for t in ent[1]:
                        t()
                    continue
                _, slot, tkp = ent
                if slot not in oaccs:
                    oaccs[slot] = ps_o.tile([HE, QS], F32, tag="oacc",
                                            name="oacc")
                npairs = 4 * (slot + 1)
                qcols = slice(slot * QS, (slot + 1) * QS)
                use_dr = USE_DR and slot > 0
                sps = ps_s.tile([P, 2, QS], F32, tag="sT")
                for h in range(2):
                    tk = 2 * tkp + h
                    if h == 0:
                        nc.tensor.matmul(
                            sps[:, 0, :], kTlow[:, tk * P : (tk + 1) * P],
                            qTd[:H, qcols], start=True, stop=True,
                        )
                    else:
                        nc.tensor.matmul(
                            sps[:, 1, :], vkT[H:, tk * P : (tk + 1) * P],
                            qTd[H:, qcols], start=True, stop=True,
                        )
                j = tkp - 4 * slot
                if j in (0, 1) and slot > 0:
                    # diagonal triangle, additive pre-exp (overlaps exps)
                    nc.vector.tensor_tensor(
                        sps[:], sps[:], tri[:, 2 * j : 2 * j + 2, :],
                        mybir.AluOpType.add,
                    )
                expT = work.tile([P, 2, QS], EDT if use_dr else BF16,
                                 tag="expT")
                # filler block (j in 2,3): dead folds zeroed via exp bias
                fb = prm_sb[:, 2 + slot : 3 + slot] if j in (2, 3) else 0.0
                nc.scalar.activation(
                    expT[:], sps[:], mybir.ActivationFunctionType.Exp,
                    scale=float(H) ** -0.5, bias=fb,
                )
                if j in (0, 1) and slot == 0:
                    # slot 0 masks after exp: the first exps of the kernel
                    # sit on the serial head path, the DVE add would delay
                    nc.vector.tensor_tensor(
                        expT[:], expT[:], tri01[:, 2 * j : 2 * j + 2, :],
                        mybir.AluOpType.mult,
                    )
                pipe.append((expT, slot, tkp, emitted[slot] == 0,
                             emitted[slot] == npairs - 1))
                emitted[slot] += 1
                if len(pipe) > 1:
                    pop_wv()
            while pipe:
                pop_wv()
            flush_out(pending)

    nc.compile()
    return nc


_NC_CACHE = None


def _get_nc():
    global _NC_CACHE
    if _NC_CACHE is None:
        _NC_CACHE = build_bass()
    return _NC_CACHE


def _core_inputs(x, Wq, bq, Wk, bk, Wv, bv, b, fold):
    xT = np.asarray(x[b], dtype=np.float32).T       # [C, T] view
    perm = EM_PERM[fold]
    xa = np.empty((TB, P, CB, QS), dtype=BF16NP)
    for em, blk in enumerate(perm):
        t = xT[:, blk * QS : (blk + 1) * QS].reshape(CB, P, QS)
        xa[em] = t.transpose(1, 0, 2)

    wv = np.asarray(Wv, np.float32).reshape(CB, P, H)
    wk = np.asarray(Wk, np.float32).reshape(CB, P, H)
    w_vk = np.concatenate([wv, wk], axis=2).transpose(1, 0, 2)  # [P, CB, 128]
    wq1 = np.asarray(Wq, np.float32).reshape(CB, P, H)
    w_q = np.concatenate([wq1, wq1], axis=2).transpose(1, 0, 2)

    prm = np.zeros((P, 8), dtype=np.float32)
    prm[:H, 0] = np.asarray(bv, np.float32)
    prm[H:, 0] = np.asarray(bk, np.float32)
    prm[:H, 1] = np.asarray(bq, np.float32)
    prm[H:, 1] = np.asarray(bq, np.float32)
    for s in range(NSLOT):
        if perm[2 * s + 1] > perm[2 * s]:   # filler block is fully dead
            prm[:, 2 + s] = -100.0
    return {
        "x_all": np.ascontiguousarray(xa),
        "w_vk": np.ascontiguousarray(w_vk.astype(BF16NP)),
        "w_q": np.ascontiguousarray(w_q.astype(BF16NP)),
        "prm": prm,
    }


def _unshard(results):
    out = np.empty((B, T, H), dtype=np.float32)
    for core in range(8):
        b, fold = core // 2, core % 2
        o = results[core]["out"]            # [NSLOT, 65, 512]
        for s, q0 in enumerate(FOLD_SLOT_QSTART[fold]):
            out[b, q0 : q0 + QS, :] = (o[s, :H, :] / o[s, H : H + 1, :]).T
    return out


def kernel(x, Wq, bq, Wk, bk, Wv, bv):
    x = np.asarray(x, dtype=np.float32)
    nc = _get_nc()
    core_ids = list(range(8))
    in_maps = [
        _core_inputs(x, Wq, bq, Wk, bk, Wv, bv, core // 2, core % 2)
        for core in core_ids
    ]
    res = run_bass_kernel_spmd(nc, in_maps, core_ids)
    return _unshard(res.results)
